# revision 11
# baseline (speedup 1.0000x reference)
"""MoE routing kernel for Trainium2 (8 NeuronCores, batch-parallel).

Problem: nn_MoE_47278999994656.
  x [8, 256, 80, 80] f32 + gate Linear(256->5) + 5 experts
  (residual conv1x1 on each 128-ch half, gated by a sigmoid transform),
  top-1 masked-softmax gate => weights are EXACTLY one-hot, so
  out[b] = expert_{argmax_e logits[b,e]}(x[b]).

Sharding: data-parallel over batch, core i computes batch item i.

Per core: x is pre-cast to bf16 on the host (it is only ever consumed as
bf16) halving input DMA; the gate pixel-sums are split between the DVE
(half 0, free-axis reduces) and the PE (half 1, PSUM-accumulated matmuls)
so both finish with the load; the selected expert's weights come from a
one-hot-mask weighted sum.  The expert runs as a software-pipelined chunk
loop: D via bf16 matmuls with fused (I+W) residual, H partition-stacked,
A computed as a [33,n] matmul (rows 0/32 live => ONE sigmoid per chunk
covers both halves) and broadcast back to 128 partitions by stride-0-source DMAs (sync + SWDGE
rings); PSUM evacuations are split across Vector/Scalar (gpsimd cannot
read PSUM) and the final combine runs as bf16 tensor_tensor ops at the
2x DVE rate with the add alternating onto gpsimd.
Output is written bf16 and widened to f32 on the host.
"""

import numpy as np

import concourse.bacc as bacc_mod
import concourse.bass as bass
import concourse.mybir as mybir
import concourse.tile as tile
from concourse.bass_utils import run_bass_kernel_spmd

B, C, H, W = 8, 256, 80, 80
HW = H * W          # 6400
HALF = 128
QUARTER = 64
E = 5
NCORES = 8

# expert-layer chunks: 12 x 512 + 1 x 256
CHUNKS = [(i * 512, 512) for i in range(12)] + [(6144, 256)]
NCH = len(CHUNKS)
# x DMA chunks per half
XCH0 = [(0, 1600), (1600, 1600), (3200, 1600), (4800, 1200), (6000, 400)]
XCH1 = [(0, 1600), (1600, 1600), (3200, 1600), (4800, 1600)]

# U_all free-dim layout (per expert, partition dim = 128):
#   [0:128)    (I + Wrgb)^T        [c, o]
#   [128:256)  (I + Wtir)^T        [c, o]
#   [256:320)  Wt1^T               [o, m]   (m = 64)
#   [320:353)  Wt2 cols: col0=[Wt2;0], col32=[0;Wt2], rest zero
#              (33 cols so sig_r lands on partition 0, sig_t on partition 32,
#               both legal partition_broadcast sources)
UF = 353
U_RGB = 0
U_TIR = 128
U_WT1 = 256
U_WT2 = 320

F32 = mybir.dt.float32
BF16 = mybir.dt.bfloat16
AX = mybir.AxisListType.X
OP = mybir.AluOpType
AF = mybir.ActivationFunctionType


def build_nc() -> bass.Bass:
    nc = bacc_mod.Bacc()

    x_d = nc.dram_tensor("x", [C, HW], BF16, kind="ExternalInput")
    u_d = nc.dram_tensor("u", [HALF, E, UF], BF16, kind="ExternalInput")
    bias_d = nc.dram_tensor("bias", [HALF, E, 4], F32, kind="ExternalInput")
    wgb_d = nc.dram_tensor("wgb", [HALF, E], BF16, kind="ExternalInput")
    wgf_d = nc.dram_tensor("wgf", [HALF, E], F32, kind="ExternalInput")
    bg_d = nc.dram_tensor("bg", [1, E], F32, kind="ExternalInput")
    out_d = nc.dram_tensor("out", [HALF, HW], BF16, kind="ExternalOutput")

    with tile.TileContext(nc) as tc:
        with (
            tc.tile_pool(name="big", bufs=1) as big,
            tc.tile_pool(name="const", bufs=1) as const,
            tc.tile_pool(name="small", bufs=1) as small,
            tc.tile_pool(name="sb3", bufs=3) as sb3,
            tc.tile_pool(name="dps", bufs=3, space="PSUM") as dps,
            tc.tile_pool(name="hps", bufs=2, space="PSUM") as hps,
            tc.tile_pool(name="aps", bufs=2, space="PSUM") as aps,
            tc.tile_pool(name="jps", bufs=1, space="PSUM") as jps,
        ):
            # ---- persistent SBUF ----
            xb = big.tile([HALF, 2, HW], BF16)
            dsb = big.tile([HALF, 2, HW], BF16)
            osb = big.tile([HALF, HW], BF16)
            u_all = const.tile([HALF, E, UF], BF16)
            bias_all = const.tile([HALF, E, 4], F32)
            wgb = const.tile([HALF, E], BF16)
            wgf = const.tile([HALF, E], F32)
            bgx = const.tile([1, E], F32)
            ones1 = const.tile([1, HALF], F32)
            t32a = small.tile([32, 32], F32)

            nc.scalar.dma_start(out=u_all[:], in_=u_d[:])
            nc.scalar.dma_start(out=bias_all[:], in_=bias_d[:])
            nc.scalar.dma_start(out=wgb[:], in_=wgb_d[:])
            nc.scalar.dma_start(out=wgf[:], in_=wgf_d[:])
            nc.scalar.dma_start(out=bgx[:], in_=bg_d[:])
            nc.vector.memset(ones1, 1.0)
            nc.vector.memset(t32a, 0.0)

            # ---- phase 1: x load (bf16), gate split V/PE ----
            ndma = max(len(XCH0), len(XCH1))
            for k in range(ndma):
                if k < len(XCH1):
                    o, n = XCH1[k]
                    nc.sync.dma_start(
                        out=xb[:, 1, o : o + n], in_=x_d[HALF:C, o : o + n]
                    )
                if k < len(XCH0):
                    o, n = XCH0[k]
                    nc.sync.dma_start(
                        out=xb[:, 0, o : o + n], in_=x_d[0:HALF, o : o + n]
                    )

            # half-1 gate matmuls into Y [5,512] (+ junk mms for PE warmth)
            yg = dps.tile([E, 512], F32, tag="d")
            for j, (off, n) in enumerate(CHUNKS):
                nc.tensor.matmul(
                    yg[:, 0:n], lhsT=wgb[:], rhs=xb[:, 1, off : off + n],
                    start=(j == 0), stop=(j == NCH - 1),
                )
                jnk = jps.tile([E, 512], F32, tag="j", name=f"jnk{j}")
                nc.tensor.matmul(jnk[:, 0:n], lhsT=wgb[:], rhs=xb[:, 1, off : off + n])

            # half-0 pixel-sum reduces on DVE
            p5 = small.tile([HALF, len(XCH0)], F32)
            for k, (o, n) in enumerate(XCH0):
                nc.vector.reduce_sum(p5[:, k : k + 1], xb[:, 0, o : o + n], axis=AX)
            pooled0 = small.tile([HALF, 1], F32)
            nc.vector.reduce_sum(pooled0, p5, axis=AX)
            l0 = hps.tile([E, 1], F32, tag="h")
            nc.tensor.matmul(l0, lhsT=wgf[:], rhs=pooled0)

            # Y free-reduce on scalar engine accumulator (parallel with V tail)
            l5g = small.tile([E, 1], F32)
            ydump = small.tile([E, 512], BF16)
            nc.scalar.activation(
                out=ydump, in_=yg, func=AF.Copy, accum_out=l5g
            )

            # logits -> one-hot mask row -> per-partition mask mbc [128,5]
            lsum = small.tile([E, 1], F32)
            nc.vector.tensor_tensor(out=lsum, in0=l5g, in1=l0, op=OP.add)
            nc.vector.tensor_copy(t32a[0:E, 0:1], lsum)
            t32b = small.tile([32, 32], F32)
            nc.vector.transpose(t32b, t32a)
            lrow = small.tile([1, E], F32)
            nc.vector.tensor_tensor(
                out=lrow, in0=t32b[0:1, 0:E], in1=bgx[0:1, :], op=OP.add
            )
            lmax = small.tile([1, 1], F32)
            nc.vector.reduce_max(lmax, lrow, axis=AX)
            mrow = small.tile([1, E], F32)
            nc.vector.tensor_scalar(
                out=mrow, in0=lrow, scalar1=lmax, scalar2=None, op0=OP.is_equal
            )
            mps = aps.tile([HALF, E], F32, tag="a")
            nc.tensor.matmul(mps, lhsT=ones1, rhs=mrow)
            mbc = small.tile([HALF, E], F32)
            nc.vector.tensor_copy(mbc, mps)

            # ---- select expert weights (mask exactly one-hot) ----
            # V does e0/e1 muls + the accumulate chain; S copy-scales e2..e4
            # in parallel. No in-place 3-operand ops (HW-divergence risk).
            usel = small.tile([HALF, UF], BF16)
            ut = [None] * E
            nc.vector.tensor_scalar_mul(usel, u_all[:, 0, :], mbc[:, 0:1])
            ut1 = small.tile([HALF, UF], BF16)
            nc.vector.tensor_scalar_mul(ut1, u_all[:, 1, :], mbc[:, 1:2])
            for e in range(2, E):
                ue = small.tile([HALF, UF], BF16, name=f"ut{e}")
                nc.scalar.activation(
                    out=ue, in_=u_all[:, e, :], func=AF.Copy,
                    scale=mbc[:, e : e + 1],
                )
                ut[e] = ue
                # junk matmul keeps PE p-state up through the select chain
                jnk = jps.tile([HALF, 512], F32, tag="j", name=f"jsel{e}")
                nc.tensor.matmul(
                    jnk, lhsT=u_all[:, e, 0:HALF], rhs=xb[:, 1, 0:512]
                )
            nc.vector.tensor_add(usel, usel, ut1)
            nc.vector.tensor_add(ut[2], ut[2], ut[3])
            nc.vector.tensor_add(usel, usel, ut[4])
            nc.vector.tensor_add(usel, usel, ut[2])
            bsel = small.tile([HALF, 4], F32)
            nc.scalar.activation(
                out=bsel, in_=bias_all[:, 0, :], func=AF.Copy, scale=mbc[:, 0:1]
            )
            for e in range(1, E):
                btmp = small.tile([HALF, 4], F32, name=f"btmp{e}")
                nc.scalar.activation(
                    out=btmp, in_=bias_all[:, e, :], func=AF.Copy,
                    scale=mbc[:, e : e + 1],
                )
                nc.gpsimd.tensor_add(bsel, bsel, btmp)

            # ---- phase 2: selected expert, skewed software pipeline ----
            # stages at iter k:
            #   T:  mmD(k) | mmA(k-2) | mmH(k-1)
            #   V:  stt-combine(k-5) | evacDr(k) | evacH(k-1) on odd k
            #   S:  sig(k-3) | evacDt(k) | evacH(k-1) on even k
            #   G:  partition_broadcast x2 (k-4)
            #   SP: out-dma(k-5)
            dr_t, dt_t = [None] * NCH, [None] * NCH
            h_t = [None] * NCH
            a_t = [None] * NCH
            sg_t = [None] * NCH
            srsb_t = [None] * NCH

            for k in range(NCH + 5):
                # T: D matmuls for chunk k
                if k < NCH:
                    off, n = CHUNKS[k]
                    dr = dps.tile([HALF, 512], F32, tag="d", name=f"dr{k}")
                    nc.tensor.matmul(
                        dr[:, 0:n], lhsT=usel[:, U_RGB : U_RGB + HALF],
                        rhs=xb[:, 0, off : off + n],
                    )
                    dt = dps.tile([HALF, 512], F32, tag="d", name=f"dt{k}")
                    nc.tensor.matmul(
                        dt[:, 0:n], lhsT=usel[:, U_TIR : U_TIR + HALF],
                        rhs=xb[:, 1, off : off + n],
                    )
                    dr_t[k], dt_t[k] = dr, dt

                # T: A matmul for chunk k-2 -> [33, n] (rows 0 and 32 live)
                if 2 <= k < NCH + 2:
                    c = k - 2
                    off, n = CHUNKS[c]
                    a2 = aps.tile([33, 512], F32, tag="a", name=f"a{c}")
                    nc.tensor.matmul(
                        a2[:, 0:n], lhsT=usel[:, U_WT2 : U_WT2 + 33],
                        rhs=h_t[c][:, 0:n],
                    )
                    a_t[c] = a2

                # T: H matmuls for chunk k-1 (rhs = evac'd D)
                if 1 <= k < NCH + 1:
                    c = k - 1
                    off, n = CHUNKS[c]
                    hp = hps.tile([HALF, 512], F32, tag="h", name=f"h{c}")
                    nc.tensor.matmul(
                        hp[0:QUARTER, 0:n],
                        lhsT=usel[:, U_WT1 : U_WT1 + QUARTER],
                        rhs=dsb[:, 0, off : off + n],
                    )
                    nc.tensor.matmul(
                        hp[QUARTER:HALF, 0:n],
                        lhsT=usel[:, U_WT1 : U_WT1 + QUARTER],
                        rhs=dsb[:, 1, off : off + n],
                        tile_position=(0, QUARTER),
                    )
                    h_t[c] = hp

                # V(+G): combine for chunk k-5 (bf16 tensor_tensor at 2x)
                if 5 <= k and k - 5 < NCH:
                    c = k - 5
                    off, n = CHUNKS[c]
                    srsb = srsb_t[c]
                    prt = sb3.tile([HALF, 512], BF16, tag="prt", name=f"prt{c}")
                    nc.vector.tensor_mul(
                        prt[:, 0:n], dsb[:, 0, off : off + n], srsb[:, 0, 0:n]
                    )
                    ot = sb3.tile([HALF, 512], BF16, tag="ot", name=f"ot{c}")
                    nc.vector.tensor_mul(
                        ot[:, 0:n], dsb[:, 1, off : off + n], srsb[:, 1, 0:n]
                    )
                    eng = nc.gpsimd if c % 2 == 1 else nc.vector
                    eng.tensor_add(
                        osb[:, off : off + n], prt[:, 0:n], ot[:, 0:n]
                    )
                    # paired output stores (fewer DMA issues on the sync ring)
                    if c % 2 == 1:
                        po, pn = CHUNKS[c - 1][0], CHUNKS[c - 1][1] + n
                        nc.sync.dma_start(
                            out=out_d[:, po : po + pn], in_=osb[:, po : po + pn]
                        )
                    elif c == NCH - 1:
                        nc.sync.dma_start(
                            out=out_d[:, off : off + n], in_=osb[:, off : off + n]
                        )

                # S: sigmoid for chunk k-3 (ONE [33,n] op covers both rows)
                if 3 <= k < NCH + 3:
                    c = k - 3
                    off, n = CHUNKS[c]
                    sg = sb3.tile([33, 512], BF16, tag="sg", name=f"sg{c}")
                    nc.scalar.activation(
                        out=sg[:, 0:n], in_=a_t[c][:, 0:n],
                        func=AF.Sigmoid, bias=bsel[0:33, 3:4],
                    )
                    sg_t[c] = sg

                # DMA: broadcast sig rows 0/32 to all 128 partitions
                # (stride-0 source APs; one on the sync ring, one on SWDGE)
                if 4 <= k < NCH + 4:
                    c = k - 4
                    off, n = CHUNKS[c]
                    srsb = sb3.tile([HALF, 2, 512], BF16, tag="srsb",
                                    name=f"srsb{c}")
                    nc.sync.dma_start(
                        out=srsb[:, 0, 0:n],
                        in_=sg_t[c][0:1, 0:n].unsqueeze(1).broadcast_to(
                            (1, HALF, n)
                        ),
                    )
                    nc.gpsimd.dma_start(
                        out=srsb[:, 1, 0:n],
                        in_=sg_t[c][32:33, 0:n].unsqueeze(1).broadcast_to(
                            (1, HALF, n)
                        ),
                    )
                    srsb_t[c] = srsb

                # V/S: evac D for chunk k (bias add, f32 PSUM -> bf16 SBUF)
                if k < NCH:
                    off, n = CHUNKS[k]
                    nc.vector.tensor_scalar_add(
                        dsb[:, 0, off : off + n], dr_t[k][:, 0:n], bsel[:, 0:1]
                    )
                    nc.scalar.activation(
                        out=dsb[:, 1, off : off + n], in_=dt_t[k][:, 0:n],
                        func=AF.Identity, bias=bsel[:, 1:2],
                    )

                # evac H for chunk k-1 (bias + relu), alternating V/S
                if 1 <= k < NCH + 1:
                    c = k - 1
                    off, n = CHUNKS[c]
                    hsb = sb3.tile([HALF, 512], BF16, tag="hsb", name=f"hsb{c}")
                    if c % 4 == 0:
                        nc.vector.tensor_scalar(
                            out=hsb[:, 0:n], in0=h_t[c][:, 0:n],
                            scalar1=bsel[:, 2:3], scalar2=0.0,
                            op0=OP.add, op1=OP.max,
                        )
                    else:
                        nc.scalar.activation(
                            out=hsb[:, 0:n], in_=h_t[c][:, 0:n],
                            func=AF.Relu, bias=bsel[:, 2:3],
                        )
                    h_t[c] = hsb

    nc.compile()
    return nc


def _pack_inputs(x, Wg, bg, Wrgb, brgb, Wtir, btir, Wt1, bt1, Wt2, bt2):
    import ml_dtypes
    eye = np.eye(HALF, dtype=np.float32)
    u = np.zeros((E, HALF, UF), dtype=np.float32)
    for e in range(E):
        u[e, :, U_RGB : U_RGB + HALF] = Wrgb[e].T + eye
        u[e, :, U_TIR : U_TIR + HALF] = Wtir[e].T + eye
        u[e, :, U_WT1 : U_WT1 + QUARTER] = Wt1[e].T
        u[e, 0:QUARTER, U_WT2] = Wt2[e, 0]
        u[e, QUARTER:HALF, U_WT2 + 32] = Wt2[e, 0]
    u = np.ascontiguousarray(u.transpose(1, 0, 2)).astype(ml_dtypes.bfloat16)

    bias = np.zeros((E, HALF, 4), dtype=np.float32)
    for e in range(E):
        bias[e, :, 0] = brgb[e]
        bias[e, :, 1] = btir[e]
        bias[e, 0:QUARTER, 2] = bt1[e]
        bias[e, QUARTER:HALF, 2] = bt1[e]
        bias[e, :, 3] = bt2[e, 0]
    bias = np.ascontiguousarray(bias.transpose(1, 0, 2))

    wgt = Wg.T.astype(np.float32)                   # [256, 5]
    wgb = np.ascontiguousarray(wgt[HALF:]).astype(ml_dtypes.bfloat16)
    wgf = np.ascontiguousarray(wgt[:HALF])
    bgx = np.ascontiguousarray((bg * float(HW))[None, :].astype(np.float32))

    common = {"u": u, "bias": bias, "wgb": wgb, "wgf": wgf, "bg": bgx}
    in_maps = []
    for b in range(B):
        m = dict(common)
        m["x"] = np.ascontiguousarray(
            x[b].reshape(C, HW).astype(ml_dtypes.bfloat16)
        )
        in_maps.append(m)
    return in_maps


_NC_CACHE = {}


def _get_nc():
    if "nc" not in _NC_CACHE:
        _NC_CACHE["nc"] = build_nc()
    return _NC_CACHE["nc"]


def kernel(x, Wg, bg, Wrgb, brgb, Wtir, btir, Wt1, bt1, Wt2, bt2, **run_kw):
    nc = _get_nc()
    in_maps = _pack_inputs(
        np.asarray(x), np.asarray(Wg), np.asarray(bg), np.asarray(Wrgb),
        np.asarray(brgb), np.asarray(Wtir), np.asarray(btir),
        np.asarray(Wt1), np.asarray(bt1), np.asarray(Wt2), np.asarray(bt2),
    )
    res = run_bass_kernel_spmd(nc, in_maps, core_ids=list(range(NCORES)), **run_kw)
    out = np.stack(
        [np.asarray(r["out"]).astype(np.float32) for r in res.results], axis=0
    )
    if run_kw:
        kernel.last_results = res
    return out.reshape(B, HALF, H, W)
